# revision 24
# baseline (speedup 1.0000x reference)
"""Trainium2 Bass kernel for batched scaled-dot-product attention + 1x1-conv FFN.

Reference computation (per batch n of 4):
    S    = q @ k.T / 8           [P, P]   (P=4096, d_k=64)
    A    = softmax(S, axis=-1)
    out  = (A @ v) @ W.T + b     [P, 256]

Sharding: 8 cores = 4 batches x 2 query-halves (2048 queries each, full K/V).
No collectives needed; host scatters inputs / gathers outputs.

Per-core dataflow (flash-attention style, query tiles of 512, all matmuls
bf16 with fp32 PSUM accumulation):
    - S^T chunks [128kv, 512q] via ROW-TILED TensorE matmuls: the d_k=64
      contraction only needs half the PE array, so even kv chunks run on
      the 64x128 tile at rows 0-63 and odd chunks on rows 64-127
      concurrently (host packs kT pairs into the two partition halves and
      ships qT duplicated into both halves) -> 2 chunks per 512-cycle
      stream instead of 1
    - exp on ScalarE, PSUM -> SBUF bf16, scale=1/8 fused into the
      activation; no max subtraction needed (scores/8 ~ N(0,1))
    - A @ [V | 1]: exp^T chunks as the stationary operand over V augmented
      with a ones column, so the softmax denominator falls out of the same
      PSUM accumulation; deferred normalization
    - per-partition reciprocal + scale on VectorE, attn^T via xbar
      DMA-transpose, FC against host-pretransposed W^T (two subtiles'
      FC outputs share one PSUM bank to limit pool-rotation coupling),
      bias added on VectorE with bf16 output (host casts back to fp32)
The software pipeline keeps TensorE ~97% busy: S/exp are emitted two
pairs ahead of the A@V bursts and scheduled at priority 0, so each
tile's S pairs overlap the previous tile's AV tail; the previous tile's
transpose/FC/store epilogue is spread through the current tile's steady
loop; 11 warmup matmul pairs bridge the initial DMA wait so the PE HAM
clock-gate opens (2.4 GHz) before the real matmuls; input DMAs are
issued in consumption order (fine chops first, coarse blocks later,
scalar queue kept free for exp); final stores are column-chopped across
the three DMA-capable queues to shorten the end-of-kernel drain.
"""

import sys

sys.path.insert(0, "/opt/trn_rl_repo")

from contextlib import ExitStack

import ml_dtypes
import numpy as np

import concourse.tile as tile
from concourse import bacc, mybir
from concourse.masks import make_identity

N_BATCH = 4
P_KV = 4096  # keys/values per batch
D_K = 64
D_V = 256
N_CORES = 8
Q_SHARD = N_BATCH * P_KV // N_CORES  # 2048 queries per core
QT = 512  # query tile width
N_QT = Q_SHARD // QT  # 4
N_SUB = QT // 128  # 4 query sub-tiles per tile
N_KC = P_KV // 128  # 32 kv chunks
N_PAIR = N_KC // 2  # 16 chunk pairs (even on PE rows 0-63, odd on 64-127)

F32 = mybir.dt.float32
BF16 = mybir.dt.bfloat16


def build_nc():
    nc = bacc.Bacc("TRN2", target_bir_lowering=False, debug=False)
    # Host-prepped layouts (bf16):
    #   qt [128, Q_SHARD]: rows 0-63 = q.T, rows 64-127 = the same q.T again
    #     (the odd-chunk row-tile streams its moving operand from the upper
    #     partition half)
    #   kt [128, P_KV/2]: column block p (128 wide) holds chunk 2p's k.T in
    #     rows 0-63 and chunk 2p+1's k.T in rows 64-127
    q_d = nc.declare_dram_parameter("qt", [128, Q_SHARD], BF16, isOutput=False)
    k_d = nc.declare_dram_parameter("kt", [128, P_KV // 2], BF16, isOutput=False)
    v_d = nc.declare_dram_parameter("v", [P_KV, D_V], BF16, isOutput=False)
    w_d = nc.declare_dram_parameter("wt", [D_V, D_V], BF16, isOutput=False)
    b_d = nc.declare_dram_parameter("b", [D_V], F32, isOutput=False)
    o_d = nc.declare_dram_parameter("out", [Q_SHARD, D_V], BF16, isOutput=True)

    with tile.TileContext(nc) as tc, ExitStack() as ctx:
        persist = ctx.enter_context(tc.tile_pool(name="persist", bufs=1))
        stage = ctx.enter_context(tc.tile_pool(name="stage", bufs=1))
        sb_small = ctx.enter_context(tc.tile_pool(name="small", bufs=4))
        sb_attn = ctx.enter_context(tc.tile_pool(name="attn", bufs=6))
        sb_out = ctx.enter_context(tc.tile_pool(name="osb", bufs=6))
        sb_exp = ctx.enter_context(tc.tile_pool(name="exp", bufs=8))
        # PSUM: ps_s = 2 x [128,1024] (2 banks each) shared by S^T chunks and
        # the FC outputs; ps_o = 4 x [128,264] (1 bank each) for the 4
        # per-subtile attention accumulators. Total 8 banks.
        ps_s = ctx.enter_context(tc.tile_pool(name="ps_s", bufs=2, space="PSUM"))
        ps_o = ctx.enter_context(tc.tile_pool(name="ps_o", bufs=4, space="PSUM"))

        # ---- persistent tiles ----
        kt_t = persist.tile([128, P_KV // 2], BF16, tag="kt", name="kt")
        qTs = [
            persist.tile([128, QT], BF16, tag=f"qT{tq}", name=f"qT{tq}")
            for tq in range(N_QT)
        ]
        v_aug = persist.tile([128, N_KC, D_V + 8], BF16, tag="v_aug")
        wT = persist.tile([128, 2, D_V], BF16, tag="wT")
        b_nat = persist.tile([1, D_V], F32, tag="b_nat")
        b_bcast2 = persist.tile([128, 2 * D_V], F32, tag="b_bcast2")
        ident = persist.tile([128, 128], F32, tag="ident")
        identb = persist.tile([128, 128], BF16, tag="identb")
        warm = stage.tile([128, QT], BF16, tag="warm")

        # ---- PE warmup bridge ----
        # memset on gpsimd (its queue is free first), then row-tiled garbage
        # matmul pairs keep the PE busy through the initial DMA wait so HAM
        # un-throttles to 2.4 GHz before the real S matmuls begin.
        nc.gpsimd.memset(warm, 0.0)
        for _ in range(10):
            pw = ps_s.tile([128, 2 * QT], F32, tag="s", name="pw")
            nc.tensor.matmul(
                pw[:, 0:QT], lhsT=warm[0:64, 0:128], rhs=warm[0:64, :],
                start=True, stop=True,
            )
            nc.tensor.matmul(
                pw[:, QT : 2 * QT], lhsT=warm[64:128, 0:128], rhs=warm[64:128, :],
                start=True, stop=True,
            )

        # ---- staged input DMAs, in consumption order ----
        # Only sync/scalar/gpsimd can issue DMAs, each issue costs the engine
        # ~0.65us, and each DMA lands on its own ~22GB/s queue. So: fine
        # chops (<=64KB) for first-needed data, ~128KB blocks later, issued
        # round-robin in deadline order. Scalar gets only 2 early issues so
        # the exp table load stays early.
        v_re = v_d[:].rearrange("(c p) v -> p c v", p=128)

        def ld(eng, dst, src):
            eng.dma_start(out=dst, in_=src)

        # wave 1: first S pair (qt cols 0-511, kt cols 0-127) + v chunk 0
        ld(nc.sync, qTs[0][:, 0:128], q_d[:, 0:128])
        ld(nc.gpsimd, qTs[0][:, 128:256], q_d[:, 128:256])
        ld(nc.scalar, qTs[0][:, 256:384], q_d[:, 256:384])
        ld(nc.sync, qTs[0][:, 384:512], q_d[:, 384:512])
        ld(nc.gpsimd, kt_t[:, 0:128], k_d[:, 0:128])
        ld(nc.scalar, kt_t[:, 128:256], k_d[:, 128:256])
        ld(nc.sync, v_aug[:, 0, 0:D_V], v_re[:, 0, :])
        # wave 2: per-chunk v (tight deadlines) interleaved with kt
        ld(nc.gpsimd, kt_t[:, 256:384], k_d[:, 256:384])
        ld(nc.sync, kt_t[:, 384:512], k_d[:, 384:512])
        ld(nc.gpsimd, v_aug[:, 1, 0:D_V], v_re[:, 1, :])
        ld(nc.sync, v_aug[:, 2, 0:D_V], v_re[:, 2, :])
        ld(nc.sync, v_aug[:, 3, 0:D_V], v_re[:, 3, :])
        ld(nc.gpsimd, v_aug[:, 4, 0:D_V], v_re[:, 4, :])
        ld(nc.sync, kt_t[:, 512:768], k_d[:, 512:768])
        ld(nc.gpsimd, v_aug[:, 5, 0:D_V], v_re[:, 5, :])
        ld(nc.sync, v_aug[:, 6, 0:D_V], v_re[:, 6, :])
        ld(nc.gpsimd, v_aug[:, 7, 0:D_V], v_re[:, 7, :])
        ld(nc.sync, kt_t[:, 768:1024], k_d[:, 768:1024])
        ld(nc.gpsimd, v_aug[:, 8, 0:D_V], v_re[:, 8, :])
        ld(nc.sync, v_aug[:, 9, 0:D_V], v_re[:, 9, :])
        # wave 3: 2-chunk v blocks + rest of kt + late tiles
        ld(nc.gpsimd, kt_t[:, 1024:1408], k_d[:, 1024:1408])
        ld(nc.sync, v_aug[:, 10:12, 0:D_V], v_re[:, 10:12, :])
        ld(nc.gpsimd, v_aug[:, 12:14, 0:D_V], v_re[:, 12:14, :])
        ld(nc.sync, kt_t[:, 1408:2048], k_d[:, 1408:2048])
        ld(nc.gpsimd, v_aug[:, 14:16, 0:D_V], v_re[:, 14:16, :])
        ld(nc.sync, v_aug[:, 16:18, 0:D_V], v_re[:, 16:18, :])
        ld(nc.gpsimd, qTs[1], q_d[:, QT : 2 * QT])
        ld(nc.sync, v_aug[:, 18:20, 0:D_V], v_re[:, 18:20, :])
        ld(nc.gpsimd, v_aug[:, 20:22, 0:D_V], v_re[:, 20:22, :])
        ld(nc.sync, wT, w_d[:].rearrange("(cc p) o -> p cc o", p=128))
        ld(nc.gpsimd, v_aug[:, 22:24, 0:D_V], v_re[:, 22:24, :])
        ld(nc.sync, v_aug[:, 24:26, 0:D_V], v_re[:, 24:26, :])
        ld(nc.gpsimd, v_aug[:, 26:28, 0:D_V], v_re[:, 26:28, :])
        ld(nc.sync, b_nat, b_d[:].unsqueeze(0))
        ld(nc.gpsimd, v_aug[:, 28:30, 0:D_V], v_re[:, 28:30, :])
        ld(nc.sync, v_aug[:, 30:32, 0:D_V], v_re[:, 30:32, :])
        ld(nc.gpsimd, qTs[2], q_d[:, 2 * QT : 3 * QT])
        ld(nc.sync, qTs[3], q_d[:, 3 * QT : 4 * QT])

        # ---- constants ----
        nc.vector.memset(v_aug[:, :, D_V : D_V + 8], 1.0)
        make_identity(nc, ident)
        nc.vector.tensor_copy(identb, ident)
        nc.gpsimd.partition_broadcast(b_bcast2[:, 0:D_V], b_nat)
        nc.vector.tensor_copy(b_bcast2[:, D_V : 2 * D_V], b_bcast2[:, 0:D_V])

        # ---- main loop over query tiles ----
        # The transpose+FC+out epilogue of tile t-1 is spread inside tile t's
        # steady loop so PE fills exp-wait gaps instead of a serial tail.
        tailstate = {}

        def emit_transpose(attn, qt_prev, s, pe_transpose=False):
            key = (qt_prev, s)
            attnT = sb_attn.tile([128, 2, 128], BF16, tag="att", name="attnT")
            tailstate[key] = attnT
            if pe_transpose:
                for cc in range(2):
                    pt = ps_s.tile([128, 128], BF16, tag="s", name=f"pt{cc}")
                    nc.tensor.transpose(
                        pt, attn[:, cc * 128 : (cc + 1) * 128], identb
                    )
                    nc.vector.tensor_copy(attnT[:, cc, :], pt)
            else:
                nc.sync.dma_start(out=attnT, in_=attn, transpose=True)

        def emit_fc_pair(qt_prev, sp, fin=False):
            # FC for subtiles 2sp, 2sp+1 into ONE [128,512] PSUM tile: halves
            # the ps_s rotation disruptions that would otherwise put S pairs
            # behind FC matmuls
            pf = ps_s.tile([128, 2 * D_V], F32, tag="s", name="pf")
            for si in range(2):
                s = 2 * sp + si
                attnT = tailstate.pop((qt_prev, s))
                for cc in range(2):
                    nc.tensor.matmul(
                        pf[:, si * D_V : (si + 1) * D_V],
                        lhsT=(attnT[:, cc, :]),
                        rhs=(wT[:, cc, :]),
                        start=(cc == 0),
                        stop=(cc == 1),
                    )
            osb = sb_out.tile([128, 2 * D_V], BF16, tag="ou", name="osb")
            nc.vector.tensor_add(osb, pf, b_bcast2)
            for si in range(2):
                s = 2 * sp + si
                row0 = qt_prev * QT + s * 128
                src = osb[:, si * D_V : (si + 1) * D_V]
                if fin:
                    # final stores: chop columns across queues (keeps all 128
                    # SBUF partitions feeding each DMA) so the drain is short
                    third = 96
                    nc.sync.dma_start(
                        out=o_d[row0 : row0 + 128, 0:third], in_=src[:, 0:third]
                    )
                    nc.scalar.dma_start(
                        out=o_d[row0 : row0 + 128, third : 2 * third],
                        in_=src[:, third : 2 * third],
                    )
                    nc.gpsimd.dma_start(
                        out=o_d[row0 : row0 + 128, 2 * third : D_V],
                        in_=src[:, 2 * third : D_V],
                    )
                else:
                    nc.gpsimd.dma_start(out=o_d[row0 : row0 + 128, :], in_=src)

        prev = None
        pending = []
        for qt in range(N_QT):
            for attn_p, po_p, recip_p in pending:
                nc.vector.tensor_scalar_mul(attn_p, po_p[:, 0:D_V], recip_p)
            pending = []
            po = [
                ps_o.tile([128, D_V + 8], F32, tag="o", name=f"po{s}")
                for s in range(N_SUB)
            ]
            expTs = {}

            def emit_s_exp(idx2):
                # chunk pair idx2: even chunk on PE rows 0-63, odd chunk on
                # rows 64-127, streamed concurrently (2x row tiling)
                jj = 2 * idx2
                pcol = idx2 * 128
                ps = ps_s.tile([128, 2 * QT], F32, tag="s", name="ps")
                expT = sb_exp.tile([128, 2 * QT], BF16, tag="expT", name="expT")
                expTs[jj] = expT
                nc.tensor.matmul(
                    ps[:, 0:QT],
                    lhsT=kt_t[0:64, pcol : pcol + 128],
                    rhs=qTs[qt][0:64, :],
                    start=True,
                    stop=True,
                )
                nc.tensor.matmul(
                    ps[:, QT : 2 * QT],
                    lhsT=kt_t[64:128, pcol : pcol + 128],
                    rhs=qTs[qt][64:128, :],
                    start=True,
                    stop=True,
                )
                nc.scalar.activation(
                    out=expT[:, :],
                    in_=ps[:, :],
                    func=mybir.ActivationFunctionType.Exp,
                    scale=0.125,
                )

            def emit_s_exp_early(idx2):
                # schedule S/exp as early as dependencies allow (priority 0):
                # at tile boundaries the next tile's S pairs then overlap the
                # current tile's AV tail instead of queueing behind it, which
                # removes a ~1us PE bubble + 3us ScalarE stall per boundary
                with tc.high_priority():
                    emit_s_exp(idx2)

            # S/exp run TWO pairs ahead of AV (the S pair's PSUM slot is
            # freed by the ACT two pairs back, so 2-ahead is legal)
            emit_s_exp_early(0)
            emit_s_exp_early(1)
            for idx in range(N_PAIR):
                if idx + 2 < N_PAIR:
                    emit_s_exp_early(idx + 2)
                if prev is not None:
                    p_attns, p_qt = prev
                    if 2 <= idx < 2 + N_SUB:
                        emit_transpose(p_attns[idx - 2], p_qt, idx - 2)
                    elif idx == 8 or idx == 12:
                        emit_fc_pair(p_qt, (idx - 8) // 4)
                jj = 2 * idx
                for dj in range(2):
                    j = jj + dj
                    for s in range(N_SUB):
                        nc.tensor.matmul(
                            po[s],
                            lhsT=expTs[jj][
                                :, dj * QT + s * 128 : dj * QT + (s + 1) * 128
                            ],
                            rhs=(v_aug[:, j, :]),
                            start=(j == 0),
                            stop=(j == N_KC - 1),
                        )

            attns = []
            for s in range(N_SUB):
                recip = sb_small.tile([128, 1], F32, tag="rc", name="recip")
                nc.vector.reciprocal(recip, po[s][:, D_V : D_V + 1])
                attn = sb_attn.tile([128, D_V], BF16, tag="at", name="attn")
                if qt == N_QT - 1 and s % 2 == 1:
                    nc.scalar.activation(
                        out=attn,
                        in_=po[s][:, 0:D_V],
                        func=mybir.ActivationFunctionType.Copy,
                        scale=recip,
                    )
                elif qt < N_QT - 1 and s >= 2:
                    pending.append((attn, po[s], recip))
                else:
                    nc.vector.tensor_scalar_mul(attn, po[s][:, 0:D_V], recip)
                attns.append(attn)
            prev = (attns, qt)

        # last tile epilogue: transposes for a pair, then its FC while the
        # next pair's transposes run — first stores issue earlier
        p_attns, p_qt = prev
        emit_transpose(p_attns[0], p_qt, 0, pe_transpose=True)
        emit_transpose(p_attns[1], p_qt, 1, pe_transpose=True)
        emit_fc_pair(p_qt, 0, fin=True)
        emit_transpose(p_attns[2], p_qt, 2, pe_transpose=True)
        emit_transpose(p_attns[3], p_qt, 3, pe_transpose=True)
        emit_fc_pair(p_qt, 1, fin=True)

    nc.compile()
    return nc


_NC_CACHE = None


def _get_nc():
    global _NC_CACHE
    if _NC_CACHE is None:
        _NC_CACHE = build_nc()
    return _NC_CACHE


def _pack_kt(k):
    # [4096, 64] -> [128, 2048]: pair p cols 128p..128p+127 hold chunk 2p's
    # k.T in rows 0-63 and chunk 2p+1's k.T in rows 64-127
    k = np.asarray(k).astype(ml_dtypes.bfloat16)
    kk = k.reshape(N_PAIR, 2, 128, D_K)  # [pair, parity, 128kv, 64]
    top = np.transpose(kk[:, 0], (2, 0, 1)).reshape(D_K, N_PAIR * 128)
    bot = np.transpose(kk[:, 1], (2, 0, 1)).reshape(D_K, N_PAIR * 128)
    return np.ascontiguousarray(np.concatenate([top, bot], axis=0))


def _dup_qt(q):
    qt = np.asarray(q).T.astype(ml_dtypes.bfloat16)  # [64, Q_SHARD]
    return np.ascontiguousarray(np.concatenate([qt, qt], axis=0))


def make_in_maps(k_src, v_src, q_tgr, W_fc, b_fc):
    in_maps = []
    for core in range(N_CORES):
        n, h = divmod(core, 2)
        in_maps.append(
            {
                "qt": _dup_qt(q_tgr[n, h * Q_SHARD : (h + 1) * Q_SHARD, :]),
                "kt": _pack_kt(k_src[n]),
                "v": np.ascontiguousarray(np.asarray(v_src[n]).astype(ml_dtypes.bfloat16)),
                "wt": np.ascontiguousarray(
                    np.asarray(W_fc).T.astype(ml_dtypes.bfloat16)
                ),
                "b": np.ascontiguousarray(b_fc, dtype=np.float32),
            }
        )
    return in_maps


def assemble_out(results):
    out = np.empty((N_BATCH, P_KV, D_V), dtype=np.float32)
    for core in range(N_CORES):
        n, h = divmod(core, 2)
        out[n, h * Q_SHARD : (h + 1) * Q_SHARD, :] = np.asarray(
            results[core]["out"]
        ).astype(np.float32)
    return out


def kernel(k_src, v_src, q_tgr, W_fc, b_fc):
    from concourse.bass_utils import run_bass_kernel_spmd

    nc = _get_nc()
    in_maps = make_in_maps(k_src, v_src, q_tgr, W_fc, b_fc)
    res = run_bass_kernel_spmd(nc, in_maps, core_ids=list(range(N_CORES)))
    return assemble_out(res.results)


# revision 25
# speedup vs baseline: 1.0091x; 1.0091x over previous
"""Trainium2 Bass kernel for batched scaled-dot-product attention + 1x1-conv FFN.

Reference computation (per batch n of 4):
    S    = q @ k.T / 8           [P, P]   (P=4096, d_k=64)
    A    = softmax(S, axis=-1)
    out  = (A @ v) @ W.T + b     [P, 256]

Sharding: 8 cores = 4 batches x 2 query-halves (2048 queries each, full K/V).
No collectives needed; host scatters inputs / gathers outputs.

Per-core dataflow (flash-attention style, query tiles of 512, all matmuls
bf16 with fp32 PSUM accumulation):
    - S^T chunks [128kv, 512q] via ROW-TILED TensorE matmuls: the d_k=64
      contraction only needs half the PE array, so even kv chunks run on
      the 64x128 tile at rows 0-63 and odd chunks on rows 64-127
      concurrently (host packs kT pairs into the two partition halves and
      ships qT duplicated into both halves) -> 2 chunks per 512-cycle
      stream instead of 1
    - exp on ScalarE, PSUM -> SBUF bf16, scale=1/8 fused into the
      activation; no max subtraction needed (scores/8 ~ N(0,1))
    - A @ [V | 1]: exp^T chunks as the stationary operand over V augmented
      with a ones column, so the softmax denominator falls out of the same
      PSUM accumulation; deferred normalization
    - per-partition reciprocal + scale on VectorE, attn^T via xbar
      DMA-transpose, FC against host-pretransposed W^T (two subtiles'
      FC outputs share one PSUM bank to limit pool-rotation coupling),
      bias added on VectorE with bf16 output (host casts back to fp32)
The software pipeline keeps TensorE ~97% busy: S/exp are emitted two
pairs ahead of the A@V bursts and scheduled at priority 0, so each
tile's S pairs overlap the previous tile's AV tail; the previous tile's
transpose/FC/store epilogue is spread through the current tile's steady
loop; 11 warmup matmul pairs bridge the initial DMA wait so the PE HAM
clock-gate opens (2.4 GHz) before the real matmuls; input DMAs are
issued in consumption order (fine chops first, coarse blocks later,
scalar queue kept free for exp); final stores are column-chopped across
the three DMA-capable queues to shorten the end-of-kernel drain.
"""

import sys

sys.path.insert(0, "/opt/trn_rl_repo")

from contextlib import ExitStack

import ml_dtypes
import numpy as np

import concourse.tile as tile
from concourse import bacc, mybir
from concourse.masks import make_identity

N_BATCH = 4
P_KV = 4096  # keys/values per batch
D_K = 64
D_V = 256
N_CORES = 8
Q_SHARD = N_BATCH * P_KV // N_CORES  # 2048 queries per core
QT = 512  # query tile width
N_QT = Q_SHARD // QT  # 4
N_SUB = QT // 128  # 4 query sub-tiles per tile
N_KC = P_KV // 128  # 32 kv chunks
N_PAIR = N_KC // 2  # 16 chunk pairs (even on PE rows 0-63, odd on 64-127)

F32 = mybir.dt.float32
BF16 = mybir.dt.bfloat16


def build_nc():
    nc = bacc.Bacc("TRN2", target_bir_lowering=False, debug=False)
    # Host-prepped layouts (bf16):
    #   qt [128, Q_SHARD]: rows 0-63 = q.T, rows 64-127 = the same q.T again
    #     (the odd-chunk row-tile streams its moving operand from the upper
    #     partition half)
    #   kt [128, P_KV/2]: column block p (128 wide) holds chunk 2p's k.T in
    #     rows 0-63 and chunk 2p+1's k.T in rows 64-127
    q_d = nc.declare_dram_parameter("qt", [128, Q_SHARD], BF16, isOutput=False)
    k_d = nc.declare_dram_parameter("kt", [128, P_KV // 2], BF16, isOutput=False)
    v_d = nc.declare_dram_parameter("v", [P_KV, D_V], BF16, isOutput=False)
    w_d = nc.declare_dram_parameter("wt", [D_V, D_V], BF16, isOutput=False)
    b_d = nc.declare_dram_parameter("b", [D_V], F32, isOutput=False)
    o_d = nc.declare_dram_parameter("out", [Q_SHARD, D_V], BF16, isOutput=True)

    with tile.TileContext(nc) as tc, ExitStack() as ctx:
        persist = ctx.enter_context(tc.tile_pool(name="persist", bufs=1))
        stage = ctx.enter_context(tc.tile_pool(name="stage", bufs=1))
        sb_small = ctx.enter_context(tc.tile_pool(name="small", bufs=4))
        sb_attn = ctx.enter_context(tc.tile_pool(name="attn", bufs=6))
        sb_out = ctx.enter_context(tc.tile_pool(name="osb", bufs=6))
        sb_exp = ctx.enter_context(tc.tile_pool(name="exp", bufs=8))
        # PSUM: ps_s = 2 x [128,1024] (2 banks each) shared by S^T chunks and
        # the FC outputs; ps_o = 4 x [128,264] (1 bank each) for the 4
        # per-subtile attention accumulators. Total 8 banks.
        ps_s = ctx.enter_context(tc.tile_pool(name="ps_s", bufs=2, space="PSUM"))
        ps_o = ctx.enter_context(tc.tile_pool(name="ps_o", bufs=4, space="PSUM"))

        # ---- persistent tiles ----
        kt_t = persist.tile([128, P_KV // 2], BF16, tag="kt", name="kt")
        qTs = [
            persist.tile([128, QT], BF16, tag=f"qT{tq}", name=f"qT{tq}")
            for tq in range(N_QT)
        ]
        v_aug = persist.tile([128, N_KC, D_V + 8], BF16, tag="v_aug")
        wT = persist.tile([128, 2, D_V], BF16, tag="wT")
        b_nat = persist.tile([1, D_V], F32, tag="b_nat")
        b_bcast2 = persist.tile([128, 2 * D_V], F32, tag="b_bcast2")
        ident = persist.tile([128, 128], F32, tag="ident")
        identb = persist.tile([128, 128], BF16, tag="identb")
        warm = stage.tile([128, QT], BF16, tag="warm")

        # ---- PE warmup bridge ----
        # memset on gpsimd (its queue is free first), then row-tiled garbage
        # matmul pairs keep the PE busy through the initial DMA wait so HAM
        # un-throttles to 2.4 GHz before the real S matmuls begin.
        nc.gpsimd.memset(warm, 0.0)
        for _ in range(10):
            pw = ps_s.tile([128, 2 * QT], F32, tag="s", name="pw")
            nc.tensor.matmul(
                pw[:, 0:QT], lhsT=warm[0:64, 0:128], rhs=warm[0:64, :],
                start=True, stop=True,
            )
            nc.tensor.matmul(
                pw[:, QT : 2 * QT], lhsT=warm[64:128, 0:128], rhs=warm[64:128, :],
                start=True, stop=True,
            )

        # ---- staged input DMAs, in consumption order ----
        # Only sync/scalar/gpsimd can issue DMAs, each issue costs the engine
        # ~0.65us, and each DMA lands on its own ~22GB/s queue. So: fine
        # chops (<=64KB) for first-needed data, ~128KB blocks later, issued
        # round-robin in deadline order. Scalar gets only 2 early issues so
        # the exp table load stays early.
        v_re = v_d[:].rearrange("(c p) v -> p c v", p=128)

        def ld(eng, dst, src):
            eng.dma_start(out=dst, in_=src)

        # wave 1: first S pair (qt cols 0-511, kt cols 0-127) + v chunk 0
        ld(nc.sync, qTs[0][:, 0:128], q_d[:, 0:128])
        ld(nc.gpsimd, qTs[0][:, 128:256], q_d[:, 128:256])
        ld(nc.scalar, qTs[0][:, 256:384], q_d[:, 256:384])
        ld(nc.sync, qTs[0][:, 384:512], q_d[:, 384:512])
        ld(nc.gpsimd, kt_t[:, 0:128], k_d[:, 0:128])
        ld(nc.scalar, kt_t[:, 128:256], k_d[:, 128:256])
        ld(nc.sync, v_aug[:, 0, 0:D_V], v_re[:, 0, :])
        # wave 2: per-chunk v (tight deadlines) interleaved with kt
        ld(nc.gpsimd, kt_t[:, 256:384], k_d[:, 256:384])
        ld(nc.sync, kt_t[:, 384:512], k_d[:, 384:512])
        ld(nc.gpsimd, v_aug[:, 1, 0:D_V], v_re[:, 1, :])
        ld(nc.sync, v_aug[:, 2, 0:D_V], v_re[:, 2, :])
        ld(nc.sync, v_aug[:, 3, 0:D_V], v_re[:, 3, :])
        ld(nc.gpsimd, v_aug[:, 4, 0:D_V], v_re[:, 4, :])
        ld(nc.sync, kt_t[:, 512:768], k_d[:, 512:768])
        ld(nc.gpsimd, v_aug[:, 5, 0:D_V], v_re[:, 5, :])
        ld(nc.sync, v_aug[:, 6, 0:D_V], v_re[:, 6, :])
        ld(nc.gpsimd, v_aug[:, 7, 0:D_V], v_re[:, 7, :])
        ld(nc.sync, kt_t[:, 768:1024], k_d[:, 768:1024])
        ld(nc.gpsimd, v_aug[:, 8, 0:D_V], v_re[:, 8, :])
        ld(nc.sync, v_aug[:, 9, 0:D_V], v_re[:, 9, :])
        # wave 3: 2-chunk v blocks + rest of kt + late tiles
        ld(nc.gpsimd, kt_t[:, 1024:1408], k_d[:, 1024:1408])
        ld(nc.sync, v_aug[:, 10:12, 0:D_V], v_re[:, 10:12, :])
        ld(nc.gpsimd, v_aug[:, 12:14, 0:D_V], v_re[:, 12:14, :])
        ld(nc.sync, kt_t[:, 1408:2048], k_d[:, 1408:2048])
        ld(nc.gpsimd, v_aug[:, 14:16, 0:D_V], v_re[:, 14:16, :])
        ld(nc.sync, v_aug[:, 16:18, 0:D_V], v_re[:, 16:18, :])
        ld(nc.gpsimd, qTs[1], q_d[:, QT : 2 * QT])
        ld(nc.sync, v_aug[:, 18:20, 0:D_V], v_re[:, 18:20, :])
        ld(nc.gpsimd, v_aug[:, 20:22, 0:D_V], v_re[:, 20:22, :])
        ld(nc.sync, wT, w_d[:].rearrange("(cc p) o -> p cc o", p=128))
        ld(nc.gpsimd, v_aug[:, 22:24, 0:D_V], v_re[:, 22:24, :])
        ld(nc.sync, v_aug[:, 24:26, 0:D_V], v_re[:, 24:26, :])
        ld(nc.gpsimd, v_aug[:, 26:28, 0:D_V], v_re[:, 26:28, :])
        ld(nc.sync, b_nat, b_d[:].unsqueeze(0))
        ld(nc.gpsimd, v_aug[:, 28:30, 0:D_V], v_re[:, 28:30, :])
        ld(nc.sync, v_aug[:, 30:32, 0:D_V], v_re[:, 30:32, :])
        ld(nc.gpsimd, qTs[2], q_d[:, 2 * QT : 3 * QT])
        ld(nc.sync, qTs[3], q_d[:, 3 * QT : 4 * QT])

        # ---- constants ----
        nc.vector.memset(v_aug[:, :, D_V : D_V + 8], 1.0)
        make_identity(nc, ident)
        nc.vector.tensor_copy(identb, ident)
        nc.gpsimd.partition_broadcast(b_bcast2[:, 0:D_V], b_nat)
        nc.vector.tensor_copy(b_bcast2[:, D_V : 2 * D_V], b_bcast2[:, 0:D_V])

        # ---- main loop over query tiles ----
        # The transpose+FC+out epilogue of tile t-1 is spread inside tile t's
        # steady loop so PE fills exp-wait gaps instead of a serial tail.
        tailstate = {}

        def emit_transpose(attn, qt_prev, s, pe_transpose=False):
            key = (qt_prev, s)
            attnT = sb_attn.tile([128, 2, 128], BF16, tag="att", name="attnT")
            tailstate[key] = attnT
            if pe_transpose:
                for cc in range(2):
                    pt = ps_s.tile([128, 128], BF16, tag="s", name=f"pt{cc}")
                    nc.tensor.transpose(
                        pt, attn[:, cc * 128 : (cc + 1) * 128], identb
                    )
                    nc.vector.tensor_copy(attnT[:, cc, :], pt)
            else:
                nc.sync.dma_start(out=attnT, in_=attn, transpose=True)

        def emit_fc_pair(qt_prev, sp, fin=False):
            # FC for subtiles 2sp, 2sp+1 into ONE [128,512] PSUM tile: halves
            # the ps_s rotation disruptions that would otherwise put S pairs
            # behind FC matmuls
            pf = ps_s.tile([128, 2 * D_V], F32, tag="s", name="pf")
            for si in range(2):
                s = 2 * sp + si
                attnT = tailstate.pop((qt_prev, s))
                for cc in range(2):
                    nc.tensor.matmul(
                        pf[:, si * D_V : (si + 1) * D_V],
                        lhsT=(attnT[:, cc, :]),
                        rhs=(wT[:, cc, :]),
                        start=(cc == 0),
                        stop=(cc == 1),
                    )
            osb = sb_out.tile([128, 2 * D_V], BF16, tag="ou", name="osb")
            if fin:
                # split the bias-add so the first subtile's stores issue
                # before the second half is evacuated
                nc.vector.tensor_add(
                    osb[:, 0:D_V], pf[:, 0:D_V], b_bcast2[:, 0:D_V]
                )
                nc.vector.tensor_add(
                    osb[:, D_V : 2 * D_V], pf[:, D_V : 2 * D_V], b_bcast2[:, 0:D_V]
                )
            else:
                nc.vector.tensor_add(osb, pf, b_bcast2)
            for si in range(2):
                s = 2 * sp + si
                row0 = qt_prev * QT + s * 128
                src = osb[:, si * D_V : (si + 1) * D_V]
                if fin:
                    # final stores: chop columns across queues (keeps all 128
                    # SBUF partitions feeding each DMA) so the drain is short
                    third = 96
                    nc.sync.dma_start(
                        out=o_d[row0 : row0 + 128, 0:third], in_=src[:, 0:third]
                    )
                    nc.scalar.dma_start(
                        out=o_d[row0 : row0 + 128, third : 2 * third],
                        in_=src[:, third : 2 * third],
                    )
                    nc.gpsimd.dma_start(
                        out=o_d[row0 : row0 + 128, 2 * third : D_V],
                        in_=src[:, 2 * third : D_V],
                    )
                else:
                    nc.gpsimd.dma_start(out=o_d[row0 : row0 + 128, :], in_=src)

        prev = None
        pending = []
        for qt in range(N_QT):
            for attn_p, po_p, recip_p in pending:
                nc.vector.tensor_scalar_mul(attn_p, po_p[:, 0:D_V], recip_p)
            pending = []
            po = [
                ps_o.tile([128, D_V + 8], F32, tag="o", name=f"po{s}")
                for s in range(N_SUB)
            ]
            expTs = {}

            def emit_s_exp(idx2):
                # chunk pair idx2: even chunk on PE rows 0-63, odd chunk on
                # rows 64-127, streamed concurrently (2x row tiling)
                jj = 2 * idx2
                pcol = idx2 * 128
                ps = ps_s.tile([128, 2 * QT], F32, tag="s", name="ps")
                expT = sb_exp.tile([128, 2 * QT], BF16, tag="expT", name="expT")
                expTs[jj] = expT
                nc.tensor.matmul(
                    ps[:, 0:QT],
                    lhsT=kt_t[0:64, pcol : pcol + 128],
                    rhs=qTs[qt][0:64, :],
                    start=True,
                    stop=True,
                )
                nc.tensor.matmul(
                    ps[:, QT : 2 * QT],
                    lhsT=kt_t[64:128, pcol : pcol + 128],
                    rhs=qTs[qt][64:128, :],
                    start=True,
                    stop=True,
                )
                nc.scalar.activation(
                    out=expT[:, :],
                    in_=ps[:, :],
                    func=mybir.ActivationFunctionType.Exp,
                    scale=0.125,
                )

            def emit_s_exp_early(idx2):
                # schedule S/exp as early as dependencies allow (priority 0):
                # at tile boundaries the next tile's S pairs then overlap the
                # current tile's AV tail instead of queueing behind it, which
                # removes a ~1us PE bubble + 3us ScalarE stall per boundary
                with tc.high_priority():
                    emit_s_exp(idx2)

            # S/exp run TWO pairs ahead of AV (the S pair's PSUM slot is
            # freed by the ACT two pairs back, so 2-ahead is legal)
            emit_s_exp_early(0)
            emit_s_exp_early(1)
            for idx in range(N_PAIR):
                if idx + 2 < N_PAIR:
                    emit_s_exp_early(idx + 2)
                if prev is not None:
                    p_attns, p_qt = prev
                    if 2 <= idx < 2 + N_SUB:
                        emit_transpose(p_attns[idx - 2], p_qt, idx - 2)
                    elif idx == 8 or idx == 12:
                        emit_fc_pair(p_qt, (idx - 8) // 4)
                jj = 2 * idx
                for dj in range(2):
                    j = jj + dj
                    for s in range(N_SUB):
                        nc.tensor.matmul(
                            po[s],
                            lhsT=expTs[jj][
                                :, dj * QT + s * 128 : dj * QT + (s + 1) * 128
                            ],
                            rhs=(v_aug[:, j, :]),
                            start=(j == 0),
                            stop=(j == N_KC - 1),
                        )

            attns = []
            for s in range(N_SUB):
                recip = sb_small.tile([128, 1], F32, tag="rc", name="recip")
                nc.vector.reciprocal(recip, po[s][:, D_V : D_V + 1])
                attn = sb_attn.tile([128, D_V], BF16, tag="at", name="attn")
                if qt == N_QT - 1 and s % 2 == 1:
                    nc.scalar.activation(
                        out=attn,
                        in_=po[s][:, 0:D_V],
                        func=mybir.ActivationFunctionType.Copy,
                        scale=recip,
                    )
                elif qt < N_QT - 1 and s >= 2:
                    pending.append((attn, po[s], recip))
                else:
                    nc.vector.tensor_scalar_mul(attn, po[s][:, 0:D_V], recip)
                attns.append(attn)
            prev = (attns, qt)

        # last tile epilogue: transposes for a pair, then its FC while the
        # next pair's transposes run — first stores issue earlier
        p_attns, p_qt = prev
        emit_transpose(p_attns[0], p_qt, 0, pe_transpose=True)
        emit_transpose(p_attns[1], p_qt, 1, pe_transpose=True)
        emit_fc_pair(p_qt, 0, fin=True)
        emit_transpose(p_attns[2], p_qt, 2, pe_transpose=True)
        emit_transpose(p_attns[3], p_qt, 3, pe_transpose=True)
        emit_fc_pair(p_qt, 1, fin=True)

    nc.compile()
    return nc


_NC_CACHE = None


def _get_nc():
    global _NC_CACHE
    if _NC_CACHE is None:
        _NC_CACHE = build_nc()
    return _NC_CACHE


def _pack_kt(k):
    # [4096, 64] -> [128, 2048]: pair p cols 128p..128p+127 hold chunk 2p's
    # k.T in rows 0-63 and chunk 2p+1's k.T in rows 64-127
    k = np.asarray(k).astype(ml_dtypes.bfloat16)
    kk = k.reshape(N_PAIR, 2, 128, D_K)  # [pair, parity, 128kv, 64]
    top = np.transpose(kk[:, 0], (2, 0, 1)).reshape(D_K, N_PAIR * 128)
    bot = np.transpose(kk[:, 1], (2, 0, 1)).reshape(D_K, N_PAIR * 128)
    return np.ascontiguousarray(np.concatenate([top, bot], axis=0))


def _dup_qt(q):
    qt = np.asarray(q).T.astype(ml_dtypes.bfloat16)  # [64, Q_SHARD]
    return np.ascontiguousarray(np.concatenate([qt, qt], axis=0))


def make_in_maps(k_src, v_src, q_tgr, W_fc, b_fc):
    in_maps = []
    for core in range(N_CORES):
        n, h = divmod(core, 2)
        in_maps.append(
            {
                "qt": _dup_qt(q_tgr[n, h * Q_SHARD : (h + 1) * Q_SHARD, :]),
                "kt": _pack_kt(k_src[n]),
                "v": np.ascontiguousarray(np.asarray(v_src[n]).astype(ml_dtypes.bfloat16)),
                "wt": np.ascontiguousarray(
                    np.asarray(W_fc).T.astype(ml_dtypes.bfloat16)
                ),
                "b": np.ascontiguousarray(b_fc, dtype=np.float32),
            }
        )
    return in_maps


def assemble_out(results):
    out = np.empty((N_BATCH, P_KV, D_V), dtype=np.float32)
    for core in range(N_CORES):
        n, h = divmod(core, 2)
        out[n, h * Q_SHARD : (h + 1) * Q_SHARD, :] = np.asarray(
            results[core]["out"]
        ).astype(np.float32)
    return out


def kernel(k_src, v_src, q_tgr, W_fc, b_fc):
    from concourse.bass_utils import run_bass_kernel_spmd

    nc = _get_nc()
    in_maps = make_in_maps(k_src, v_src, q_tgr, W_fc, b_fc)
    res = run_bass_kernel_spmd(nc, in_maps, core_ids=list(range(N_CORES)))
    return assemble_out(res.results)


# revision 26
# speedup vs baseline: 1.0234x; 1.0142x over previous
"""Trainium2 Bass kernel for batched scaled-dot-product attention + 1x1-conv FFN.

Reference computation (per batch n of 4):
    S    = q @ k.T / 8           [P, P]   (P=4096, d_k=64)
    A    = softmax(S, axis=-1)
    out  = (A @ v) @ W.T + b     [P, 256]

Sharding: 8 cores = 4 batches x 2 query-halves (2048 queries each, full K/V).
No collectives needed; host scatters inputs / gathers outputs.

Per-core dataflow (flash-attention style, query tiles of 512, all matmuls
bf16 with fp32 PSUM accumulation):
    - S^T chunks [128kv, 512q] via ROW-TILED TensorE matmuls: the d_k=64
      contraction only needs half the PE array, so even kv chunks run on
      the 64x128 tile at rows 0-63 and odd chunks on rows 64-127
      concurrently (host packs kT pairs into the two partition halves and
      ships qT duplicated into both halves) -> 2 chunks per 512-cycle
      stream instead of 1
    - exp on ScalarE, PSUM -> SBUF bf16, scale=1/8 fused into the
      activation; no max subtraction needed (scores/8 ~ N(0,1))
    - A @ [V | 1]: exp^T chunks as the stationary operand over V augmented
      with a ones column, so the softmax denominator falls out of the same
      PSUM accumulation; deferred normalization
    - per-partition reciprocal + scale on VectorE, attn^T via xbar
      DMA-transpose, FC against host-pretransposed W^T (two subtiles'
      FC outputs share one PSUM bank to limit pool-rotation coupling),
      bias added on VectorE with bf16 output (host casts back to fp32)
The software pipeline keeps TensorE ~97% busy: S/exp are emitted two
pairs ahead of the A@V bursts and scheduled at priority 0, so each
tile's S pairs overlap the previous tile's AV tail; the previous tile's
transpose/FC/store epilogue is spread through the current tile's steady
loop; 11 warmup matmul pairs bridge the initial DMA wait so the PE HAM
clock-gate opens (2.4 GHz) before the real matmuls; input DMAs are
issued in consumption order (fine chops first, coarse blocks later,
scalar queue kept free for exp); final stores are column-chopped across
the three DMA-capable queues to shorten the end-of-kernel drain.
"""

import sys

sys.path.insert(0, "/opt/trn_rl_repo")

from contextlib import ExitStack

import ml_dtypes
import numpy as np

import concourse.tile as tile
from concourse import bacc, mybir
from concourse.masks import make_identity

N_BATCH = 4
P_KV = 4096  # keys/values per batch
D_K = 64
D_V = 256
N_CORES = 8
Q_SHARD = N_BATCH * P_KV // N_CORES  # 2048 queries per core
QT = 512  # query tile width
N_QT = Q_SHARD // QT  # 4
N_SUB = QT // 128  # 4 query sub-tiles per tile
N_KC = P_KV // 128  # 32 kv chunks
N_PAIR = N_KC // 2  # 16 chunk pairs (even on PE rows 0-63, odd on 64-127)

F32 = mybir.dt.float32
BF16 = mybir.dt.bfloat16


def build_nc():
    nc = bacc.Bacc("TRN2", target_bir_lowering=False, debug=False)
    # Host-prepped layouts (bf16):
    #   qt [128, Q_SHARD]: rows 0-63 = q.T, rows 64-127 = the same q.T again
    #     (the odd-chunk row-tile streams its moving operand from the upper
    #     partition half)
    #   kt [128, P_KV/2]: column block p (128 wide) holds chunk 2p's k.T in
    #     rows 0-63 and chunk 2p+1's k.T in rows 64-127
    q_d = nc.declare_dram_parameter("qt", [128, Q_SHARD], BF16, isOutput=False)
    k_d = nc.declare_dram_parameter("kt", [128, P_KV // 2], BF16, isOutput=False)
    v_d = nc.declare_dram_parameter("v", [P_KV, D_V], BF16, isOutput=False)
    w_d = nc.declare_dram_parameter("wt", [D_V, D_V], BF16, isOutput=False)
    b_d = nc.declare_dram_parameter("b", [D_V], F32, isOutput=False)
    o_d = nc.declare_dram_parameter("out", [Q_SHARD, D_V], BF16, isOutput=True)

    with tile.TileContext(nc) as tc, ExitStack() as ctx:
        persist = ctx.enter_context(tc.tile_pool(name="persist", bufs=1))
        stage = ctx.enter_context(tc.tile_pool(name="stage", bufs=1))
        sb_small = ctx.enter_context(tc.tile_pool(name="small", bufs=4))
        sb_attn = ctx.enter_context(tc.tile_pool(name="attn", bufs=6))
        sb_out = ctx.enter_context(tc.tile_pool(name="osb", bufs=6))
        sb_exp = ctx.enter_context(tc.tile_pool(name="exp", bufs=8))
        # PSUM: ps_s = 2 x [128,1024] (2 banks each) shared by S^T chunks and
        # the FC outputs; ps_o = 4 x [128,264] (1 bank each) for the 4
        # per-subtile attention accumulators. Total 8 banks.
        ps_s = ctx.enter_context(tc.tile_pool(name="ps_s", bufs=2, space="PSUM"))
        ps_o = ctx.enter_context(tc.tile_pool(name="ps_o", bufs=4, space="PSUM"))

        # ---- persistent tiles ----
        kt_t = persist.tile([128, P_KV // 2], BF16, tag="kt", name="kt")
        qTs = [
            persist.tile([128, QT], BF16, tag=f"qT{tq}", name=f"qT{tq}")
            for tq in range(N_QT)
        ]
        v_aug = persist.tile([128, N_KC, D_V + 8], BF16, tag="v_aug")
        wT = persist.tile([128, 2, D_V], BF16, tag="wT")
        b_nat = persist.tile([1, D_V], F32, tag="b_nat")
        b_bcast2 = persist.tile([128, 2 * D_V], F32, tag="b_bcast2")
        ident = persist.tile([128, 128], F32, tag="ident")
        identb = persist.tile([128, 128], BF16, tag="identb")
        warm = stage.tile([128, QT], BF16, tag="warm")

        # ---- PE warmup bridge ----
        # memset on gpsimd (its queue is free first), then row-tiled garbage
        # matmul pairs keep the PE busy through the initial DMA wait so HAM
        # un-throttles to 2.4 GHz before the real S matmuls begin.
        nc.gpsimd.memset(warm, 0.0)
        for _ in range(10):
            pw = ps_s.tile([128, 2 * QT], F32, tag="s", name="pw")
            nc.tensor.matmul(
                pw[:, 0:QT], lhsT=warm[0:64, 0:128], rhs=warm[0:64, :],
                start=True, stop=True,
            )
            nc.tensor.matmul(
                pw[:, QT : 2 * QT], lhsT=warm[64:128, 0:128], rhs=warm[64:128, :],
                start=True, stop=True,
            )

        # ---- staged input DMAs, in consumption order ----
        # Only sync/scalar/gpsimd can issue DMAs, each issue costs the engine
        # ~0.65us, and each DMA lands on its own ~22GB/s queue. So: fine
        # chops (<=64KB) for first-needed data, ~128KB blocks later, issued
        # round-robin in deadline order. Scalar gets only 2 early issues so
        # the exp table load stays early.
        v_re = v_d[:].rearrange("(c p) v -> p c v", p=128)

        def ld(eng, dst, src):
            eng.dma_start(out=dst, in_=src)

        # wave 1: first S pair (qt cols 0-511, kt cols 0-127) + v chunk 0
        ld(nc.sync, qTs[0][:, 0:128], q_d[:, 0:128])
        ld(nc.gpsimd, qTs[0][:, 128:256], q_d[:, 128:256])
        ld(nc.scalar, qTs[0][:, 256:384], q_d[:, 256:384])
        ld(nc.sync, qTs[0][:, 384:512], q_d[:, 384:512])
        ld(nc.gpsimd, kt_t[:, 0:128], k_d[:, 0:128])
        ld(nc.scalar, kt_t[:, 128:256], k_d[:, 128:256])
        ld(nc.sync, v_aug[:, 0, 0:D_V], v_re[:, 0, :])
        # wave 2: per-chunk v (tight deadlines) interleaved with kt
        ld(nc.gpsimd, kt_t[:, 256:384], k_d[:, 256:384])
        ld(nc.sync, kt_t[:, 384:512], k_d[:, 384:512])
        ld(nc.gpsimd, v_aug[:, 1, 0:D_V], v_re[:, 1, :])
        ld(nc.sync, v_aug[:, 2, 0:D_V], v_re[:, 2, :])
        ld(nc.sync, v_aug[:, 3, 0:D_V], v_re[:, 3, :])
        ld(nc.gpsimd, v_aug[:, 4, 0:D_V], v_re[:, 4, :])
        ld(nc.sync, kt_t[:, 512:768], k_d[:, 512:768])
        ld(nc.gpsimd, v_aug[:, 5, 0:D_V], v_re[:, 5, :])
        ld(nc.sync, v_aug[:, 6, 0:D_V], v_re[:, 6, :])
        ld(nc.gpsimd, v_aug[:, 7, 0:D_V], v_re[:, 7, :])
        ld(nc.sync, kt_t[:, 768:1024], k_d[:, 768:1024])
        ld(nc.gpsimd, v_aug[:, 8, 0:D_V], v_re[:, 8, :])
        ld(nc.sync, v_aug[:, 9, 0:D_V], v_re[:, 9, :])
        # wave 3: 2-chunk v blocks + rest of kt + late tiles
        ld(nc.gpsimd, kt_t[:, 1024:1408], k_d[:, 1024:1408])
        ld(nc.sync, v_aug[:, 10:12, 0:D_V], v_re[:, 10:12, :])
        ld(nc.gpsimd, v_aug[:, 12:14, 0:D_V], v_re[:, 12:14, :])
        ld(nc.sync, kt_t[:, 1408:2048], k_d[:, 1408:2048])
        ld(nc.gpsimd, v_aug[:, 14:16, 0:D_V], v_re[:, 14:16, :])
        ld(nc.sync, v_aug[:, 16:18, 0:D_V], v_re[:, 16:18, :])
        ld(nc.gpsimd, qTs[1], q_d[:, QT : 2 * QT])
        ld(nc.sync, v_aug[:, 18:20, 0:D_V], v_re[:, 18:20, :])
        ld(nc.gpsimd, v_aug[:, 20:22, 0:D_V], v_re[:, 20:22, :])
        ld(nc.sync, wT, w_d[:].rearrange("(cc p) o -> p cc o", p=128))
        ld(nc.gpsimd, v_aug[:, 22:24, 0:D_V], v_re[:, 22:24, :])
        ld(nc.sync, v_aug[:, 24:26, 0:D_V], v_re[:, 24:26, :])
        ld(nc.gpsimd, v_aug[:, 26:28, 0:D_V], v_re[:, 26:28, :])
        ld(nc.sync, b_nat, b_d[:].unsqueeze(0))
        ld(nc.gpsimd, v_aug[:, 28:30, 0:D_V], v_re[:, 28:30, :])
        ld(nc.sync, v_aug[:, 30:32, 0:D_V], v_re[:, 30:32, :])
        ld(nc.gpsimd, qTs[2], q_d[:, 2 * QT : 3 * QT])
        ld(nc.sync, qTs[3], q_d[:, 3 * QT : 4 * QT])

        # ---- constants ----
        nc.vector.memset(v_aug[:, :, D_V : D_V + 8], 1.0)
        make_identity(nc, ident)
        nc.vector.tensor_copy(identb, ident)
        nc.gpsimd.partition_broadcast(b_bcast2[:, 0:D_V], b_nat)
        nc.vector.tensor_copy(b_bcast2[:, D_V : 2 * D_V], b_bcast2[:, 0:D_V])

        # ---- main loop over query tiles ----
        # The transpose+FC+out epilogue of tile t-1 is spread inside tile t's
        # steady loop so PE fills exp-wait gaps instead of a serial tail.
        tailstate = {}

        def emit_transpose(attn, qt_prev, s, pe_transpose=False):
            key = (qt_prev, s)
            attnT = sb_attn.tile([128, 2, 128], BF16, tag="att", name="attnT")
            tailstate[key] = attnT
            if pe_transpose:
                for cc in range(2):
                    pt = ps_s.tile([128, 128], BF16, tag="s", name=f"pt{cc}")
                    nc.tensor.transpose(
                        pt, attn[:, cc * 128 : (cc + 1) * 128], identb
                    )
                    nc.vector.tensor_copy(attnT[:, cc, :], pt)
            else:
                nc.sync.dma_start(out=attnT, in_=attn, transpose=True)

        def emit_fc_pair(qt_prev, sp, fin=False):
            # FC for subtiles 2sp, 2sp+1 into ONE [128,512] PSUM tile: halves
            # the ps_s rotation disruptions that would otherwise put S pairs
            # behind FC matmuls
            pf = ps_s.tile([128, 2 * D_V], F32, tag="s", name="pf")
            for si in range(2):
                s = 2 * sp + si
                attnT = tailstate.pop((qt_prev, s))
                for cc in range(2):
                    nc.tensor.matmul(
                        pf[:, si * D_V : (si + 1) * D_V],
                        lhsT=(attnT[:, cc, :]),
                        rhs=(wT[:, cc, :]),
                        start=(cc == 0),
                        stop=(cc == 1),
                    )
            osb = sb_out.tile([128, 2 * D_V], BF16, tag="ou", name="osb")
            if fin:
                # split the bias-add so the first subtile's stores issue
                # before the second half is evacuated
                nc.vector.tensor_add(
                    osb[:, 0:D_V], pf[:, 0:D_V], b_bcast2[:, 0:D_V]
                )
                nc.vector.tensor_add(
                    osb[:, D_V : 2 * D_V], pf[:, D_V : 2 * D_V], b_bcast2[:, 0:D_V]
                )
            else:
                nc.vector.tensor_add(osb, pf, b_bcast2)
            for si in range(2):
                s = 2 * sp + si
                row0 = qt_prev * QT + s * 128
                src = osb[:, si * D_V : (si + 1) * D_V]
                if fin:
                    # final stores: chop columns across sync+scalar only —
                    # gpsimd's end-of-kernel DRAIN detects DMA completion
                    # ~2us late, so keeping it out of the last stores lets
                    # the exit barrier close earlier
                    half = D_V // 2
                    nc.sync.dma_start(
                        out=o_d[row0 : row0 + 128, 0:half], in_=src[:, 0:half]
                    )
                    nc.scalar.dma_start(
                        out=o_d[row0 : row0 + 128, half:D_V],
                        in_=src[:, half:D_V],
                    )
                else:
                    nc.gpsimd.dma_start(out=o_d[row0 : row0 + 128, :], in_=src)

        prev = None
        pending = []
        for qt in range(N_QT):
            for attn_p, po_p, recip_p in pending:
                nc.vector.tensor_scalar_mul(attn_p, po_p[:, 0:D_V], recip_p)
            pending = []
            po = [
                ps_o.tile([128, D_V + 8], F32, tag="o", name=f"po{s}")
                for s in range(N_SUB)
            ]
            expTs = {}

            def emit_s_exp(idx2):
                # chunk pair idx2: even chunk on PE rows 0-63, odd chunk on
                # rows 64-127, streamed concurrently (2x row tiling)
                jj = 2 * idx2
                pcol = idx2 * 128
                ps = ps_s.tile([128, 2 * QT], F32, tag="s", name="ps")
                expT = sb_exp.tile([128, 2 * QT], BF16, tag="expT", name="expT")
                expTs[jj] = expT
                nc.tensor.matmul(
                    ps[:, 0:QT],
                    lhsT=kt_t[0:64, pcol : pcol + 128],
                    rhs=qTs[qt][0:64, :],
                    start=True,
                    stop=True,
                )
                nc.tensor.matmul(
                    ps[:, QT : 2 * QT],
                    lhsT=kt_t[64:128, pcol : pcol + 128],
                    rhs=qTs[qt][64:128, :],
                    start=True,
                    stop=True,
                )
                nc.scalar.activation(
                    out=expT[:, :],
                    in_=ps[:, :],
                    func=mybir.ActivationFunctionType.Exp,
                    scale=0.125,
                )

            def emit_s_exp_early(idx2):
                # schedule S/exp as early as dependencies allow (priority 0):
                # at tile boundaries the next tile's S pairs then overlap the
                # current tile's AV tail instead of queueing behind it, which
                # removes a ~1us PE bubble + 3us ScalarE stall per boundary
                with tc.high_priority():
                    emit_s_exp(idx2)

            # S/exp run TWO pairs ahead of AV (the S pair's PSUM slot is
            # freed by the ACT two pairs back, so 2-ahead is legal)
            emit_s_exp_early(0)
            emit_s_exp_early(1)
            for idx in range(N_PAIR):
                if idx + 2 < N_PAIR:
                    emit_s_exp_early(idx + 2)
                if prev is not None:
                    p_attns, p_qt = prev
                    if 2 <= idx < 2 + N_SUB:
                        emit_transpose(p_attns[idx - 2], p_qt, idx - 2)
                    elif idx == 8 or idx == 12:
                        emit_fc_pair(p_qt, (idx - 8) // 4)
                jj = 2 * idx
                for dj in range(2):
                    j = jj + dj
                    for s in range(N_SUB):
                        nc.tensor.matmul(
                            po[s],
                            lhsT=expTs[jj][
                                :, dj * QT + s * 128 : dj * QT + (s + 1) * 128
                            ],
                            rhs=(v_aug[:, j, :]),
                            start=(j == 0),
                            stop=(j == N_KC - 1),
                        )

            attns = []
            for s in range(N_SUB):
                recip = sb_small.tile([128, 1], F32, tag="rc", name="recip")
                nc.vector.reciprocal(recip, po[s][:, D_V : D_V + 1])
                attn = sb_attn.tile([128, D_V], BF16, tag="at", name="attn")
                if qt == N_QT - 1 and s % 2 == 1:
                    nc.scalar.activation(
                        out=attn,
                        in_=po[s][:, 0:D_V],
                        func=mybir.ActivationFunctionType.Copy,
                        scale=recip,
                    )
                elif qt < N_QT - 1 and s >= 2:
                    pending.append((attn, po[s], recip))
                else:
                    nc.vector.tensor_scalar_mul(attn, po[s][:, 0:D_V], recip)
                attns.append(attn)
            prev = (attns, qt)

        # last tile epilogue: transposes for a pair, then its FC while the
        # next pair's transposes run — first stores issue earlier
        p_attns, p_qt = prev
        emit_transpose(p_attns[0], p_qt, 0, pe_transpose=True)
        emit_transpose(p_attns[1], p_qt, 1, pe_transpose=True)
        emit_fc_pair(p_qt, 0, fin=True)
        emit_transpose(p_attns[2], p_qt, 2, pe_transpose=True)
        emit_transpose(p_attns[3], p_qt, 3, pe_transpose=True)
        emit_fc_pair(p_qt, 1, fin=True)

    nc.compile()
    return nc


_NC_CACHE = None


def _get_nc():
    global _NC_CACHE
    if _NC_CACHE is None:
        _NC_CACHE = build_nc()
    return _NC_CACHE


def _pack_kt(k):
    # [4096, 64] -> [128, 2048]: pair p cols 128p..128p+127 hold chunk 2p's
    # k.T in rows 0-63 and chunk 2p+1's k.T in rows 64-127
    k = np.asarray(k).astype(ml_dtypes.bfloat16)
    kk = k.reshape(N_PAIR, 2, 128, D_K)  # [pair, parity, 128kv, 64]
    top = np.transpose(kk[:, 0], (2, 0, 1)).reshape(D_K, N_PAIR * 128)
    bot = np.transpose(kk[:, 1], (2, 0, 1)).reshape(D_K, N_PAIR * 128)
    return np.ascontiguousarray(np.concatenate([top, bot], axis=0))


def _dup_qt(q):
    qt = np.asarray(q).T.astype(ml_dtypes.bfloat16)  # [64, Q_SHARD]
    return np.ascontiguousarray(np.concatenate([qt, qt], axis=0))


def make_in_maps(k_src, v_src, q_tgr, W_fc, b_fc):
    in_maps = []
    for core in range(N_CORES):
        n, h = divmod(core, 2)
        in_maps.append(
            {
                "qt": _dup_qt(q_tgr[n, h * Q_SHARD : (h + 1) * Q_SHARD, :]),
                "kt": _pack_kt(k_src[n]),
                "v": np.ascontiguousarray(np.asarray(v_src[n]).astype(ml_dtypes.bfloat16)),
                "wt": np.ascontiguousarray(
                    np.asarray(W_fc).T.astype(ml_dtypes.bfloat16)
                ),
                "b": np.ascontiguousarray(b_fc, dtype=np.float32),
            }
        )
    return in_maps


def assemble_out(results):
    out = np.empty((N_BATCH, P_KV, D_V), dtype=np.float32)
    for core in range(N_CORES):
        n, h = divmod(core, 2)
        out[n, h * Q_SHARD : (h + 1) * Q_SHARD, :] = np.asarray(
            results[core]["out"]
        ).astype(np.float32)
    return out


def kernel(k_src, v_src, q_tgr, W_fc, b_fc):
    from concourse.bass_utils import run_bass_kernel_spmd

    nc = _get_nc()
    in_maps = make_in_maps(k_src, v_src, q_tgr, W_fc, b_fc)
    res = run_bass_kernel_spmd(nc, in_maps, core_ids=list(range(N_CORES)))
    return assemble_out(res.results)


# revision 28
# speedup vs baseline: 1.0396x; 1.0158x over previous
"""Trainium2 Bass kernel for batched scaled-dot-product attention + 1x1-conv FFN.

Reference computation (per batch n of 4):
    S    = q @ k.T / 8           [P, P]   (P=4096, d_k=64)
    A    = softmax(S, axis=-1)
    out  = (A @ v) @ W.T + b     [P, 256]

Sharding: 8 cores = 4 batches x 2 query-halves (2048 queries each, full K/V).
No collectives needed; host scatters inputs / gathers outputs.

Per-core dataflow (flash-attention style, query tiles of 512, all matmuls
bf16 with fp32 PSUM accumulation):
    - S^T chunks [128kv, 512q] via ROW-TILED TensorE matmuls: the d_k=64
      contraction only needs half the PE array, so even kv chunks run on
      the 64x128 tile at rows 0-63 and odd chunks on rows 64-127
      concurrently (host packs kT pairs into the two partition halves and
      ships qT duplicated into both halves) -> 2 chunks per 512-cycle
      stream instead of 1
    - exp on ScalarE, PSUM -> SBUF bf16, scale=1/8 fused into the
      activation; no max subtraction needed (scores/8 ~ N(0,1))
    - A @ [V | 1]: exp^T chunks as the stationary operand over V augmented
      with a ones column, so the softmax denominator falls out of the same
      PSUM accumulation; deferred normalization
    - per-partition reciprocal + scale on VectorE, attn^T via xbar
      DMA-transpose, FC against host-pretransposed W^T (two subtiles'
      FC outputs share one PSUM bank to limit pool-rotation coupling),
      bias added on VectorE with bf16 output (host casts back to fp32)
The software pipeline keeps TensorE ~97% busy: S/exp are emitted two
pairs ahead of the A@V bursts and scheduled at priority 0, so each
tile's S pairs overlap the previous tile's AV tail; the previous tile's
transpose/FC/store epilogue is spread through the current tile's steady
loop; 11 warmup matmul pairs bridge the initial DMA wait so the PE HAM
clock-gate opens (2.4 GHz) before the real matmuls; input DMAs are
issued in consumption order (fine chops first, coarse blocks later,
scalar queue kept free for exp); final stores are column-chopped across
the three DMA-capable queues to shorten the end-of-kernel drain.
"""

import sys

sys.path.insert(0, "/opt/trn_rl_repo")

from contextlib import ExitStack

import ml_dtypes
import numpy as np

import concourse.tile as tile
from concourse import bacc, mybir
from concourse.masks import make_identity

N_BATCH = 4
P_KV = 4096  # keys/values per batch
D_K = 64
D_V = 256
N_CORES = 8
Q_SHARD = N_BATCH * P_KV // N_CORES  # 2048 queries per core
QT = 512  # query tile width
N_QT = Q_SHARD // QT  # 4
N_SUB = QT // 128  # 4 query sub-tiles per tile
N_KC = P_KV // 128  # 32 kv chunks
N_PAIR = N_KC // 2  # 16 chunk pairs (even on PE rows 0-63, odd on 64-127)

F32 = mybir.dt.float32
BF16 = mybir.dt.bfloat16


def build_nc():
    nc = bacc.Bacc("TRN2", target_bir_lowering=False, debug=False)
    # Host-prepped layouts (bf16):
    #   qt [128, Q_SHARD]: rows 0-63 = q.T, rows 64-127 = the same q.T again
    #     (the odd-chunk row-tile streams its moving operand from the upper
    #     partition half)
    #   kt [128, P_KV/2]: column block p (128 wide) holds chunk 2p's k.T in
    #     rows 0-63 and chunk 2p+1's k.T in rows 64-127
    q_d = nc.declare_dram_parameter("qt", [128, Q_SHARD], BF16, isOutput=False)
    k_d = nc.declare_dram_parameter("kt", [128, P_KV // 2], BF16, isOutput=False)
    v_d = nc.declare_dram_parameter("v", [P_KV, D_V], BF16, isOutput=False)
    w_d = nc.declare_dram_parameter("wt", [D_V, D_V], BF16, isOutput=False)
    b_d = nc.declare_dram_parameter("b", [D_V], F32, isOutput=False)
    o_d = nc.declare_dram_parameter("out", [Q_SHARD, D_V], BF16, isOutput=True)

    with tile.TileContext(nc) as tc, ExitStack() as ctx:
        persist = ctx.enter_context(tc.tile_pool(name="persist", bufs=1))
        stage = ctx.enter_context(tc.tile_pool(name="stage", bufs=1))
        sb_small = ctx.enter_context(tc.tile_pool(name="small", bufs=4))
        sb_attn = ctx.enter_context(tc.tile_pool(name="attn", bufs=6))
        sb_out = ctx.enter_context(tc.tile_pool(name="osb", bufs=6))
        sb_exp = ctx.enter_context(tc.tile_pool(name="exp", bufs=8))
        # PSUM: ps_s = 2 x [128,1024] (2 banks each) shared by S^T chunks and
        # the FC outputs; ps_o = 4 x [128,264] (1 bank each) for the 4
        # per-subtile attention accumulators. Total 8 banks.
        ps_s = ctx.enter_context(tc.tile_pool(name="ps_s", bufs=2, space="PSUM"))
        ps_o = ctx.enter_context(tc.tile_pool(name="ps_o", bufs=4, space="PSUM"))

        # ---- persistent tiles ----
        kt_t = persist.tile([128, P_KV // 2], BF16, tag="kt", name="kt")
        qTs = [
            persist.tile([128, QT], BF16, tag=f"qT{tq}", name=f"qT{tq}")
            for tq in range(N_QT)
        ]
        v_aug = persist.tile([128, N_KC, D_V + 8], BF16, tag="v_aug")
        wT = persist.tile([128, 2, D_V], BF16, tag="wT")
        b_nat = persist.tile([1, D_V], F32, tag="b_nat")
        b_bcast2 = persist.tile([128, 2 * D_V], F32, tag="b_bcast2")
        ident = persist.tile([128, 128], F32, tag="ident")
        identb = persist.tile([128, 128], BF16, tag="identb")
        warm = stage.tile([128, QT], BF16, tag="warm")

        # ---- PE warmup bridge ----
        # memset on gpsimd (its queue is free first), then row-tiled garbage
        # matmul pairs keep the PE busy through the initial DMA wait so HAM
        # un-throttles to 2.4 GHz before the real S matmuls begin.
        nc.gpsimd.memset(warm, 0.0)
        for _ in range(10):
            pw = ps_s.tile([128, 2 * QT], F32, tag="s", name="pw")
            nc.tensor.matmul(
                pw[:, 0:QT], lhsT=warm[0:64, 0:128], rhs=warm[0:64, :],
                start=True, stop=True,
            )
            nc.tensor.matmul(
                pw[:, QT : 2 * QT], lhsT=warm[64:128, 0:128], rhs=warm[64:128, :],
                start=True, stop=True,
            )

        # ---- staged input DMAs, in consumption order ----
        # Only sync/scalar/gpsimd can issue DMAs, each issue costs the engine
        # ~0.65us, and each DMA lands on its own ~22GB/s queue. So: fine
        # chops (<=64KB) for first-needed data, ~128KB blocks later, issued
        # round-robin in deadline order. Scalar gets only 2 early issues so
        # the exp table load stays early.
        v_re = v_d[:].rearrange("(c p) v -> p c v", p=128)

        def ld(eng, dst, src):
            eng.dma_start(out=dst, in_=src)

        # wave 1: first S pair (qt cols 0-511, kt cols 0-127) + v chunk 0
        ld(nc.sync, qTs[0][:, 0:128], q_d[:, 0:128])
        ld(nc.gpsimd, qTs[0][:, 128:256], q_d[:, 128:256])
        ld(nc.scalar, qTs[0][:, 256:384], q_d[:, 256:384])
        ld(nc.sync, qTs[0][:, 384:512], q_d[:, 384:512])
        ld(nc.gpsimd, kt_t[:, 0:128], k_d[:, 0:128])
        ld(nc.scalar, kt_t[:, 128:256], k_d[:, 128:256])
        ld(nc.sync, v_aug[:, 0, 0:D_V], v_re[:, 0, :])
        # wave 2: per-chunk v (tight deadlines) interleaved with kt
        ld(nc.gpsimd, kt_t[:, 256:384], k_d[:, 256:384])
        ld(nc.sync, kt_t[:, 384:512], k_d[:, 384:512])
        ld(nc.gpsimd, v_aug[:, 1, 0:D_V], v_re[:, 1, :])
        ld(nc.sync, v_aug[:, 2, 0:D_V], v_re[:, 2, :])
        ld(nc.sync, v_aug[:, 3, 0:D_V], v_re[:, 3, :])
        ld(nc.gpsimd, v_aug[:, 4, 0:D_V], v_re[:, 4, :])
        ld(nc.sync, kt_t[:, 512:768], k_d[:, 512:768])
        ld(nc.gpsimd, v_aug[:, 5, 0:D_V], v_re[:, 5, :])
        ld(nc.sync, v_aug[:, 6, 0:D_V], v_re[:, 6, :])
        ld(nc.gpsimd, v_aug[:, 7, 0:D_V], v_re[:, 7, :])
        ld(nc.sync, kt_t[:, 768:1024], k_d[:, 768:1024])
        ld(nc.gpsimd, v_aug[:, 8, 0:D_V], v_re[:, 8, :])
        ld(nc.sync, v_aug[:, 9, 0:D_V], v_re[:, 9, :])
        # wave 3: 2-chunk v blocks + rest of kt + late tiles
        ld(nc.gpsimd, kt_t[:, 1024:1408], k_d[:, 1024:1408])
        ld(nc.sync, v_aug[:, 10:12, 0:D_V], v_re[:, 10:12, :])
        ld(nc.gpsimd, v_aug[:, 12:14, 0:D_V], v_re[:, 12:14, :])
        ld(nc.sync, kt_t[:, 1408:2048], k_d[:, 1408:2048])
        ld(nc.gpsimd, v_aug[:, 14:16, 0:D_V], v_re[:, 14:16, :])
        ld(nc.sync, v_aug[:, 16:18, 0:D_V], v_re[:, 16:18, :])
        ld(nc.gpsimd, qTs[1], q_d[:, QT : 2 * QT])
        ld(nc.sync, v_aug[:, 18:20, 0:D_V], v_re[:, 18:20, :])
        ld(nc.gpsimd, v_aug[:, 20:22, 0:D_V], v_re[:, 20:22, :])
        ld(nc.sync, wT, w_d[:].rearrange("(cc p) o -> p cc o", p=128))
        ld(nc.gpsimd, v_aug[:, 22:24, 0:D_V], v_re[:, 22:24, :])
        ld(nc.sync, v_aug[:, 24:26, 0:D_V], v_re[:, 24:26, :])
        ld(nc.gpsimd, v_aug[:, 26:28, 0:D_V], v_re[:, 26:28, :])
        ld(nc.sync, b_nat, b_d[:].unsqueeze(0))
        ld(nc.gpsimd, v_aug[:, 28:30, 0:D_V], v_re[:, 28:30, :])
        ld(nc.sync, v_aug[:, 30:32, 0:D_V], v_re[:, 30:32, :])
        ld(nc.gpsimd, qTs[2], q_d[:, 2 * QT : 3 * QT])
        ld(nc.sync, qTs[3], q_d[:, 3 * QT : 4 * QT])

        # ---- constants ----
        nc.vector.memset(v_aug[:, :, D_V : D_V + 8], 1.0)
        make_identity(nc, ident)
        nc.vector.tensor_copy(identb, ident)
        nc.gpsimd.partition_broadcast(b_bcast2[:, 0:D_V], b_nat)
        nc.vector.tensor_copy(b_bcast2[:, D_V : 2 * D_V], b_bcast2[:, 0:D_V])

        # ---- main loop over query tiles ----
        # The transpose+FC+out epilogue of tile t-1 is spread inside tile t's
        # steady loop so PE fills exp-wait gaps instead of a serial tail.
        tailstate = {}

        def emit_transpose(attn, qt_prev, s, pe_transpose=False):
            key = (qt_prev, s)
            attnT = sb_attn.tile([128, 2, 128], BF16, tag="att", name="attnT")
            tailstate[key] = attnT
            if pe_transpose:
                for cc in range(2):
                    pt = ps_s.tile([128, 128], BF16, tag="s", name=f"pt{cc}")
                    nc.tensor.transpose(
                        pt, attn[:, cc * 128 : (cc + 1) * 128], identb
                    )
                    nc.vector.tensor_copy(attnT[:, cc, :], pt)
            else:
                nc.sync.dma_start(out=attnT, in_=attn, transpose=True)

        def emit_fc_pair(qt_prev, sp, fin=False):
            # FC for subtiles 2sp, 2sp+1 into ONE [128,512] PSUM tile: halves
            # the ps_s rotation disruptions that would otherwise put S pairs
            # behind FC matmuls
            pf = ps_s.tile([128, 2 * D_V], F32, tag="s", name="pf")
            for si in range(2):
                s = 2 * sp + si
                attnT = tailstate.pop((qt_prev, s))
                for cc in range(2):
                    nc.tensor.matmul(
                        pf[:, si * D_V : (si + 1) * D_V],
                        lhsT=(attnT[:, cc, :]),
                        rhs=(wT[:, cc, :]),
                        start=(cc == 0),
                        stop=(cc == 1),
                    )
            osb = sb_out.tile([128, 2 * D_V], BF16, tag="ou", name="osb")
            if fin:
                # split the bias-add so the first subtile's stores issue
                # before the second half is evacuated
                nc.vector.tensor_add(
                    osb[:, 0:D_V], pf[:, 0:D_V], b_bcast2[:, 0:D_V]
                )
                nc.vector.tensor_add(
                    osb[:, D_V : 2 * D_V], pf[:, D_V : 2 * D_V], b_bcast2[:, 0:D_V]
                )
            else:
                nc.vector.tensor_add(osb, pf, b_bcast2)
            for si in range(2):
                s = 2 * sp + si
                row0 = qt_prev * QT + s * 128
                src = osb[:, si * D_V : (si + 1) * D_V]
                if fin:
                    # final stores: chop columns across sync+scalar only —
                    # gpsimd's end-of-kernel DRAIN detects DMA completion
                    # ~2us late, so keeping it out of the last stores lets
                    # the exit barrier close earlier
                    half = D_V // 2
                    nc.sync.dma_start(
                        out=o_d[row0 : row0 + 128, 0:half], in_=src[:, 0:half]
                    )
                    nc.scalar.dma_start(
                        out=o_d[row0 : row0 + 128, half:D_V],
                        in_=src[:, half:D_V],
                    )
                else:
                    nc.gpsimd.dma_start(out=o_d[row0 : row0 + 128, :], in_=src)

        prev = None
        pending = []
        for qt in range(N_QT):
            for attn_p, po_p, recip_p in pending:
                nc.vector.tensor_scalar_mul(attn_p, po_p[:, 0:D_V], recip_p)
            pending = []
            po = [
                ps_o.tile([128, D_V + 1], F32, tag="o", name=f"po{s}")
                for s in range(N_SUB)
            ]
            expTs = {}

            def emit_s_exp(idx2):
                # chunk pair idx2: even chunk on PE rows 0-63, odd chunk on
                # rows 64-127, streamed concurrently (2x row tiling)
                jj = 2 * idx2
                pcol = idx2 * 128
                ps = ps_s.tile([128, 2 * QT], F32, tag="s", name="ps")
                expT = sb_exp.tile([128, 2 * QT], BF16, tag="expT", name="expT")
                expTs[jj] = expT
                nc.tensor.matmul(
                    ps[:, 0:QT],
                    lhsT=kt_t[0:64, pcol : pcol + 128],
                    rhs=qTs[qt][0:64, :],
                    start=True,
                    stop=True,
                )
                nc.tensor.matmul(
                    ps[:, QT : 2 * QT],
                    lhsT=kt_t[64:128, pcol : pcol + 128],
                    rhs=qTs[qt][64:128, :],
                    start=True,
                    stop=True,
                )
                nc.scalar.activation(
                    out=expT[:, :],
                    in_=ps[:, :],
                    func=mybir.ActivationFunctionType.Exp,
                    scale=0.125,
                )

            def emit_s_exp_early(idx2):
                # schedule S/exp as early as dependencies allow (priority 0):
                # at tile boundaries the next tile's S pairs then overlap the
                # current tile's AV tail instead of queueing behind it, which
                # removes a ~1us PE bubble + 3us ScalarE stall per boundary
                with tc.high_priority():
                    emit_s_exp(idx2)

            # S/exp run TWO pairs ahead of AV (the S pair's PSUM slot is
            # freed by the ACT two pairs back, so 2-ahead is legal)
            emit_s_exp_early(0)
            emit_s_exp_early(1)
            for idx in range(N_PAIR):
                if idx + 2 < N_PAIR:
                    emit_s_exp_early(idx + 2)
                if prev is not None:
                    p_attns, p_qt = prev
                    if 2 <= idx < 2 + N_SUB:
                        emit_transpose(p_attns[idx - 2], p_qt, idx - 2)
                    elif idx == 8 or idx == 12:
                        emit_fc_pair(p_qt, (idx - 8) // 4)
                jj = 2 * idx
                for dj in range(2):
                    j = jj + dj
                    for s in range(N_SUB):
                        nc.tensor.matmul(
                            po[s],
                            lhsT=expTs[jj][
                                :, dj * QT + s * 128 : dj * QT + (s + 1) * 128
                            ],
                            rhs=(v_aug[:, j, 0 : D_V + 1]),
                            start=(j == 0),
                            stop=(j == N_KC - 1),
                        )

            attns = []
            for s in range(N_SUB):
                recip = sb_small.tile([128, 1], F32, tag="rc", name="recip")
                nc.vector.reciprocal(recip, po[s][:, D_V : D_V + 1])
                attn = sb_attn.tile([128, D_V], BF16, tag="at", name="attn")
                if qt == N_QT - 1 and s % 2 == 1:
                    nc.scalar.activation(
                        out=attn,
                        in_=po[s][:, 0:D_V],
                        func=mybir.ActivationFunctionType.Copy,
                        scale=recip,
                    )
                elif qt < N_QT - 1 and s >= 2:
                    pending.append((attn, po[s], recip))
                else:
                    nc.vector.tensor_scalar_mul(attn, po[s][:, 0:D_V], recip)
                attns.append(attn)
            prev = (attns, qt)

        # last tile epilogue: transposes for a pair, then its FC while the
        # next pair's transposes run — first stores issue earlier
        p_attns, p_qt = prev
        emit_transpose(p_attns[0], p_qt, 0, pe_transpose=True)
        emit_transpose(p_attns[1], p_qt, 1, pe_transpose=True)
        emit_fc_pair(p_qt, 0, fin=True)
        emit_transpose(p_attns[2], p_qt, 2, pe_transpose=True)
        emit_transpose(p_attns[3], p_qt, 3, pe_transpose=True)
        emit_fc_pair(p_qt, 1, fin=True)

    nc.compile()
    return nc


_NC_CACHE = None


def _get_nc():
    global _NC_CACHE
    if _NC_CACHE is None:
        _NC_CACHE = build_nc()
    return _NC_CACHE


def _pack_kt(k):
    # [4096, 64] -> [128, 2048]: pair p cols 128p..128p+127 hold chunk 2p's
    # k.T in rows 0-63 and chunk 2p+1's k.T in rows 64-127
    k = np.asarray(k).astype(ml_dtypes.bfloat16)
    kk = k.reshape(N_PAIR, 2, 128, D_K)  # [pair, parity, 128kv, 64]
    top = np.transpose(kk[:, 0], (2, 0, 1)).reshape(D_K, N_PAIR * 128)
    bot = np.transpose(kk[:, 1], (2, 0, 1)).reshape(D_K, N_PAIR * 128)
    return np.ascontiguousarray(np.concatenate([top, bot], axis=0))


def _dup_qt(q):
    qt = np.asarray(q).T.astype(ml_dtypes.bfloat16)  # [64, Q_SHARD]
    return np.ascontiguousarray(np.concatenate([qt, qt], axis=0))


def make_in_maps(k_src, v_src, q_tgr, W_fc, b_fc):
    in_maps = []
    for core in range(N_CORES):
        n, h = divmod(core, 2)
        in_maps.append(
            {
                "qt": _dup_qt(q_tgr[n, h * Q_SHARD : (h + 1) * Q_SHARD, :]),
                "kt": _pack_kt(k_src[n]),
                "v": np.ascontiguousarray(np.asarray(v_src[n]).astype(ml_dtypes.bfloat16)),
                "wt": np.ascontiguousarray(
                    np.asarray(W_fc).T.astype(ml_dtypes.bfloat16)
                ),
                "b": np.ascontiguousarray(b_fc, dtype=np.float32),
            }
        )
    return in_maps


def assemble_out(results):
    out = np.empty((N_BATCH, P_KV, D_V), dtype=np.float32)
    for core in range(N_CORES):
        n, h = divmod(core, 2)
        out[n, h * Q_SHARD : (h + 1) * Q_SHARD, :] = np.asarray(
            results[core]["out"]
        ).astype(np.float32)
    return out


def kernel(k_src, v_src, q_tgr, W_fc, b_fc):
    from concourse.bass_utils import run_bass_kernel_spmd

    nc = _get_nc()
    in_maps = make_in_maps(k_src, v_src, q_tgr, W_fc, b_fc)
    res = run_bass_kernel_spmd(nc, in_maps, core_ids=list(range(N_CORES)))
    return assemble_out(res.results)
